# revision 4
# baseline (speedup 1.0000x reference)
"""Trainium2 Bass kernel for nn_MessagePassingLayer (GNN message passing).

Strategy (edge-parallel, col-sharded, 8 cores, no collectives):
  - Host sorts edges by destination (col) and splits them into 8 contiguous
    col ranges (aligned to node boundaries), so each core owns a node range
    and all edges targeting it. Scatter-add never crosses cores.
  - Algebraic refactor: msg = relu(x[row]@Wm1a + x[col]@Wm1b + ea@Wm1c + bm1)
    aggr = segsum(msg@Wm2 + bm2) = segsum(H)@Wm2 + deg*bm2, so the per-edge
    128x128 matmul Wm2 moves to the node side:
      P[n]  = segsum_n relu(A[row] + B[col] + C_e + bm1)   (A=x@Wm1a, B=x@Wm1b)
      out   = relu(x@Wu1a + P@(Wm2@Wu1b) + deg*(bm2@Wu1b) + bu1) @ Wu2 + bu2
  - Device: pre-phase computes A (all nodes) and B (local nodes) tables in
    DRAM; edge phase gathers A[row], B[col] via indirect DMA (accumulating),
    adds C from a K=33 matmul of edge_attr^T, relu; scatter into a 128-node
    PSUM window via a one-hot selection matmul (Sel built on-chip from
    is_equal(iota, colshift)); update phase is a feature-major MLP.
  - Edges are packed per 128-node window with padding (colshift=-1 pads have
    all-zero Sel rows, contributing nothing) so the program is SPMD-uniform.
"""
import sys
sys.path.insert(0, '/opt/trn_rl_repo')

import time
import numpy as np

N_NODES = 50000
N_EDGES = 600000
NODE_IN = 128
EDGE_IN = 32
OUT_DIM = 128
N_CORES = 8
P = 128
NPAD = 50048  # N_NODES rounded up to 128


# ---------------------------------------------------------------- host prep

def _host_prep(x, edge_index, edge_attr):
    row = np.asarray(edge_index[0], dtype=np.int64)
    col = np.asarray(edge_index[1], dtype=np.int64)
    perm = np.argsort(col, kind='stable')
    row_s = row[perm].astype(np.int32)
    col_s = col[perm].astype(np.int32)
    ea_s = np.asarray(edge_attr, dtype=np.float32)[perm]

    E = row_s.shape[0]
    cuts = (np.arange(1, N_CORES) * E) // N_CORES
    nb = [0] + [int(col_s[c]) for c in cuts] + [N_NODES]
    for i in range(1, len(nb)):  # enforce nondecreasing
        nb[i] = max(nb[i], nb[i - 1])
    lo = [int(np.searchsorted(col_s, nb[k])) for k in range(N_CORES)] + [E]
    n_k = [nb[k + 1] - nb[k] for k in range(N_CORES)]
    N_LOC = max(128, int(-(-max(n_k) // 128)) * 128)
    W = N_LOC // 128

    # per-core window fill counts -> global T
    T = 1
    percore = []
    for k in range(N_CORES):
        l, h = lo[k], lo[k + 1]
        colrel = col_s[l:h] - nb[k]
        wins = colrel >> 7
        cnt = np.bincount(wins, minlength=W) if h > l else np.zeros(W, np.int64)
        T = max(T, int(-(-cnt.max() // 128)) if h > l else 1)
        percore.append((l, h, colrel, wins, cnt))
    E_LOC = W * T * 128

    xT = np.zeros((P, NPAD), np.float32)
    xT[:, :N_NODES] = np.ascontiguousarray(x.T)

    cores = []
    for k in range(N_CORES):
        l, h, colrel, wins, cnt = percore[k]
        ne = h - l
        rowidx = np.zeros(E_LOC, np.int32)
        colidx = np.zeros(E_LOC, np.int32)
        colshift = np.full(E_LOC, -1.0, np.float32)
        eaT = np.zeros((EDGE_IN + 1, E_LOC), np.float32)
        eaT[EDGE_IN, :] = 1.0
        if ne > 0:
            start = np.zeros(W, np.int64)
            start[1:] = np.cumsum(cnt)[:-1]
            within = np.arange(ne, dtype=np.int64) - start[wins]
            dst = wins.astype(np.int64) * (T * 128) + within
            rowidx[dst] = row_s[l:h]
            colidx[dst] = colrel
            colshift[dst] = (colrel - (wins << 7)).astype(np.float32)
            eaT[:EDGE_IN, dst] = ea_s[l:h].T
        deg = np.zeros(N_LOC, np.float32)
        if ne > 0:
            deg[:n_k[k]] = np.bincount(colrel, minlength=n_k[k]).astype(np.float32)[:n_k[k]]
        xT_loc = np.zeros((P, N_LOC), np.float32)
        xT_loc[:, :n_k[k]] = x.T[:, nb[k]:nb[k] + n_k[k]]
        WT = W * T
        cores.append({
            "rowidx": np.ascontiguousarray(rowidx.reshape(WT, 128).T),
            "colidx": np.ascontiguousarray(colidx.reshape(WT, 128).T),
            "colshift": np.ascontiguousarray(colshift.reshape(WT, 128).T),
            "eaT": eaT,
            "degT": deg.reshape(1, N_LOC),
            "xT_loc": xT_loc,
        })
    cfg = {"N_LOC": N_LOC, "W": W, "T": T, "E_LOC": E_LOC,
           "nb": nb, "n_k": n_k}
    return cfg, xT, cores


def _host_weights(Wm1, bm1, Wm2, bm2, Wu1, bu1, Wu2, bu2):
    Wm1 = np.asarray(Wm1, np.float32)
    Wu1 = np.asarray(Wu1, np.float32)
    w = {}
    w["wm1ab"] = np.concatenate([Wm1[0:128], Wm1[128:256]], axis=1)  # [128,256]
    w["wm1c"] = np.concatenate([Wm1[256:288], np.asarray(bm1, np.float32)[None, :]],
                               axis=0)  # [33,128]
    w["wpb"] = np.asarray(Wm2, np.float32) @ Wu1[128:256]            # [128,128]
    w["wu1a"] = np.ascontiguousarray(Wu1[0:128])                     # [128,128]
    w["wu2"] = np.asarray(Wu2, np.float32)                           # [128,128]
    vecs = np.zeros((4, 128), np.float32)
    vecs[0] = np.asarray(bm2, np.float32) @ Wu1[128:256]             # vb
    vecs[1] = np.asarray(bu1, np.float32)
    vecs[2] = np.asarray(bu2, np.float32)
    w["vecs"] = vecs
    return w


# ---------------------------------------------------------------- bass build

def _build(cfg):
    import concourse.bass as bass
    import concourse.mybir as mybir
    from concourse import bacc, tile
    from concourse.masks import make_identity

    f32 = mybir.dt.float32
    i32 = mybir.dt.int32
    Alu = mybir.AluOpType
    Act = mybir.ActivationFunctionType

    N_LOC, W, T, E_LOC = cfg["N_LOC"], cfg["W"], cfg["T"], cfg["E_LOC"]
    WT = W * T

    nc = bacc.Bacc("TRN2", target_bir_lowering=False, debug=False,
                   num_devices=N_CORES)

    xT_d = nc.dram_tensor("xT", [P, NPAD], f32, kind="ExternalInput")
    xTloc_d = nc.dram_tensor("xTloc", [P, N_LOC], f32, kind="ExternalInput")
    eaT_d = nc.dram_tensor("eaT", [EDGE_IN + 1, E_LOC], f32, kind="ExternalInput")
    rowidx_d = nc.dram_tensor("rowidx", [P, WT], i32, kind="ExternalInput")
    colidx_d = nc.dram_tensor("colidx", [P, WT], i32, kind="ExternalInput")
    colshift_d = nc.dram_tensor("colshift", [P, WT], f32, kind="ExternalInput")
    degT_d = nc.dram_tensor("degT", [1, N_LOC], f32, kind="ExternalInput")
    wm1ab_d = nc.dram_tensor("wm1ab", [P, 256], f32, kind="ExternalInput")
    wm1c_d = nc.dram_tensor("wm1c", [EDGE_IN + 1, P], f32, kind="ExternalInput")
    wpb_d = nc.dram_tensor("wpb", [P, P], f32, kind="ExternalInput")
    wu1a_d = nc.dram_tensor("wu1a", [P, P], f32, kind="ExternalInput")
    wu2_d = nc.dram_tensor("wu2", [P, P], f32, kind="ExternalInput")
    vecs_d = nc.dram_tensor("vecs", [4, P], f32, kind="ExternalInput")
    out_d = nc.dram_tensor("out", [N_LOC, P], f32, kind="ExternalOutput")

    A_d = nc.dram_tensor("Atbl", [NPAD, P], f32, kind="Internal")
    B_d = nc.dram_tensor("Btbl", [N_LOC, P], f32, kind="Internal")

    with tile.TileContext(nc) as tc:
        with tc.tile_pool(name="const", bufs=1) as cp, \
             tc.tile_pool(name="res", bufs=1) as rp:
            wm1ab = cp.tile([P, 256], f32)
            nc.sync.dma_start(out=wm1ab[:], in_=wm1ab_d[:])
            wm1c = cp.tile([EDGE_IN + 1, P], f32)
            nc.sync.dma_start(out=wm1c[:], in_=wm1c_d[:])
            wpb = cp.tile([P, P], f32)
            nc.sync.dma_start(out=wpb[:], in_=wpb_d[:])
            wu1a = cp.tile([P, P], f32)
            nc.sync.dma_start(out=wu1a[:], in_=wu1a_d[:])
            wu2 = cp.tile([P, P], f32)
            nc.sync.dma_start(out=wu2[:], in_=wu2_d[:])
            vb_sb = cp.tile([1, P], f32)
            nc.sync.dma_start(out=vb_sb[:], in_=vecs_d[0:1, :])
            bu1_sb = cp.tile([1, P], f32)
            nc.sync.dma_start(out=bu1_sb[:], in_=vecs_d[1:2, :])
            bu2_sb = cp.tile([1, P], f32)
            nc.sync.dma_start(out=bu2_sb[:], in_=vecs_d[2:3, :])
            onesrow = cp.tile([1, P], f32)
            nc.gpsimd.memset(onesrow[:], 1.0)
            ident = cp.tile([P, P], f32)
            make_identity(nc, ident[:])
            iota_i = cp.tile([P, P], i32)
            nc.gpsimd.iota(iota_i[:], [[1, P]], channel_multiplier=0)
            iota_f = cp.tile([P, P], f32)
            nc.vector.tensor_copy(out=iota_f[:], in_=iota_i[:])

            rowidx = rp.tile([P, WT], i32)
            nc.sync.dma_start(out=rowidx[:], in_=rowidx_d[:])
            colidx = rp.tile([P, WT], i32)
            nc.sync.dma_start(out=colidx[:], in_=colidx_d[:])
            colshift = rp.tile([P, WT], f32)
            nc.sync.dma_start(out=colshift[:], in_=colshift_d[:])
            xTloc = rp.tile([P, N_LOC], f32)
            nc.sync.dma_start(out=xTloc[:], in_=xTloc_d[:])
            degT = rp.tile([1, N_LOC], f32)
            nc.sync.dma_start(out=degT[:], in_=degT_d[:])
            P_loc = rp.tile([P, W * P], f32)

            # ---------------- pre phase: A (all nodes), B (local nodes)
            with tc.tile_pool(name="pre_sb", bufs=6) as pp, \
                 tc.tile_pool(name="pre_ps", bufs=4, space="PSUM") as pps:
                for i in range(NPAD // P):
                    xt = pp.tile([P, P], f32, tag="xt")
                    nc.sync.dma_start(out=xt[:], in_=xT_d[:, i * P:(i + 1) * P])
                    ps = pps.tile([P, P], f32, tag="ps")
                    nc.tensor.matmul(out=ps[:], lhsT=xt[:], rhs=wm1ab[:, 0:P],
                                     start=True, stop=True)
                    ev = pp.tile([P, P], f32, tag="ev")
                    nc.vector.tensor_copy(out=ev[:], in_=ps[:])
                    nc.sync.dma_start(out=A_d[i * P:(i + 1) * P, :], in_=ev[:])
                for i in range(W):
                    ps = pps.tile([P, P], f32, tag="ps")
                    nc.tensor.matmul(out=ps[:], lhsT=xTloc[:, i * P:(i + 1) * P],
                                     rhs=wm1ab[:, P:2 * P], start=True, stop=True)
                    ev = pp.tile([P, P], f32, tag="ev")
                    nc.vector.tensor_copy(out=ev[:], in_=ps[:])
                    nc.sync.dma_start(out=B_d[i * P:(i + 1) * P, :], in_=ev[:])

            # ---------------- edge phase
            with tc.tile_pool(name="eg_sb", bufs=6) as ep, \
                 tc.tile_pool(name="eg_ps", bufs=3, space="PSUM") as cps, \
                 tc.tile_pool(name="p_ps", bufs=2, space="PSUM") as pps2:
                groups = [4] * (T // 4) + ([T % 4] if T % 4 else [])
                for w in range(W):
                    psP = pps2.tile([P, P], f32, tag="psP")
                    tbase = w * T
                    toff = 0
                    first = True
                    for s in groups:
                        t0 = tbase + toff
                        toff += s
                        ea_sb = ep.tile([EDGE_IN + 1, 4 * P], f32, tag="ea")
                        nc.sync.dma_start(
                            out=ea_sb[:, 0:s * P],
                            in_=eaT_d[:, t0 * P:(t0 + s) * P])
                        psC = cps.tile([P, 4 * P], f32, tag="psC")
                        for j in range(s):
                            nc.tensor.matmul(out=psC[:, j * P:(j + 1) * P],
                                             lhsT=ea_sb[:, j * P:(j + 1) * P],
                                             rhs=wm1c[:], start=True, stop=True)
                        h = ep.tile([P, 4 * P], f32, tag="h")
                        for j in range(s):
                            t = t0 + j
                            nc.gpsimd.indirect_dma_start(
                                out=h[:, j * P:(j + 1) * P], out_offset=None,
                                in_=A_d[:],
                                in_offset=bass.IndirectOffsetOnAxis(
                                    ap=rowidx[:, t:t + 1], axis=0))
                            nc.gpsimd.indirect_dma_start(
                                out=h[:, j * P:(j + 1) * P], out_offset=None,
                                in_=B_d[:],
                                in_offset=bass.IndirectOffsetOnAxis(
                                    ap=colidx[:, t:t + 1], axis=0),
                                compute_op=Alu.add)
                        nc.vector.tensor_tensor(out=h[:, 0:s * P], in0=h[:, 0:s * P],
                                                in1=psC[:, 0:s * P], op=Alu.add)
                        nc.scalar.activation(out=h[:, 0:s * P], in_=h[:, 0:s * P],
                                             func=Act.Relu)
                        for j in range(s):
                            t = t0 + j
                            sel = ep.tile([P, P], f32, tag="sel")
                            nc.vector.tensor_tensor(
                                out=sel[:], in0=iota_f[:],
                                in1=colshift[:, t:t + 1].to_broadcast([P, P]),
                                op=Alu.is_equal)
                            nc.tensor.matmul(out=psP[:], lhsT=sel[:],
                                             rhs=h[:, j * P:(j + 1) * P],
                                             start=first, stop=(t == tbase + T - 1))
                            first = False
                    nc.vector.tensor_copy(out=P_loc[:, w * P:(w + 1) * P], in_=psP[:])

            # ---------------- update phase (feature-major)
            with tc.tile_pool(name="up_sb", bufs=3) as up, \
                 tc.tile_pool(name="upA_ps", bufs=2, space="PSUM") as upsA, \
                 tc.tile_pool(name="upB_ps", bufs=2, space="PSUM") as upsB, \
                 tc.tile_pool(name="upC_ps", bufs=2, space="PSUM") as upsC, \
                 tc.tile_pool(name="upD_ps", bufs=2, space="PSUM") as upsD:
                for w in range(W):
                    sl = slice(w * P, (w + 1) * P)
                    ptp = upsA.tile([P, P], f32, tag="ptp")
                    nc.tensor.transpose(out=ptp[:], in_=P_loc[:, sl], identity=ident[:])
                    pts = up.tile([P, P], f32, tag="pts")
                    nc.vector.tensor_copy(out=pts[:], in_=ptp[:])
                    ut = upsB.tile([P, P], f32, tag="ut")
                    nc.tensor.matmul(out=ut[:], lhsT=wpb[:], rhs=pts[:],
                                     start=True, stop=False)
                    nc.tensor.matmul(out=ut[:], lhsT=wu1a[:], rhs=xTloc[:, sl],
                                     start=False, stop=False)
                    nc.tensor.matmul(out=ut[:], lhsT=vb_sb[:], rhs=degT[:, sl],
                                     start=False, stop=False)
                    nc.tensor.matmul(out=ut[:], lhsT=bu1_sb[:], rhs=onesrow[:],
                                     start=False, stop=True)
                    r = up.tile([P, P], f32, tag="r")
                    nc.scalar.activation(out=r[:], in_=ut[:], func=Act.Relu)
                    o2 = upsC.tile([P, P], f32, tag="o2")
                    nc.tensor.matmul(out=o2[:], lhsT=wu2[:], rhs=r[:],
                                     start=True, stop=False)
                    nc.tensor.matmul(out=o2[:], lhsT=bu2_sb[:], rhs=onesrow[:],
                                     start=False, stop=True)
                    o2s = up.tile([P, P], f32, tag="o2s")
                    nc.vector.tensor_copy(out=o2s[:], in_=o2[:])
                    onm = upsD.tile([P, P], f32, tag="onm")
                    nc.tensor.transpose(out=onm[:], in_=o2s[:], identity=ident[:])
                    osb = up.tile([P, P], f32, tag="osb")
                    nc.vector.tensor_copy(out=osb[:], in_=onm[:])
                    nc.sync.dma_start(out=out_d[w * P:(w + 1) * P, :], in_=osb[:])

    nc.compile()
    return nc


# ---------------------------------------------------------------- runner

class SpmdRunner:
    """Jit-once PJRT runner for a prebuilt Bass module (axon path)."""

    def __init__(self, nc, n_cores):
        import jax
        from jax.sharding import Mesh, PartitionSpec
        from jax.experimental.shard_map import shard_map
        import concourse.mybir as mybir
        from concourse import bass2jax
        from concourse.bass2jax import _bass_exec_p, install_neuronx_cc_hook

        install_neuronx_cc_hook()
        self.jax = jax
        self.nc = nc
        self.n_cores = n_cores
        partition_name = nc.partition_id_tensor.name if nc.partition_id_tensor else None
        in_names, out_names, out_avals = [], [], []
        for alloc in nc.m.functions[0].allocations:
            if not isinstance(alloc, mybir.MemoryLocationSet):
                continue
            name = alloc.memorylocations[0].name
            if alloc.kind == "ExternalInput":
                if name != partition_name:
                    in_names.append(name)
            elif alloc.kind == "ExternalOutput":
                out_names.append(name)
                out_avals.append(jax.core.ShapedArray(
                    tuple(alloc.tensor_shape), mybir.dt.np(alloc.dtype)))
        self.in_names, self.out_names, self.out_avals = in_names, out_names, out_avals
        bind_in_names = list(in_names) + list(out_names)
        if partition_name is not None:
            bind_in_names.append(partition_name)

        def _body(*args):
            operands = list(args)
            if partition_name is not None:
                operands.append(bass2jax.partition_id_tensor())
            outs = _bass_exec_p.bind(
                *operands,
                out_avals=tuple(out_avals),
                in_names=tuple(bind_in_names),
                out_names=tuple(out_names),
                lowering_input_output_aliases=(),
                sim_require_finite=True,
                sim_require_nnan=True,
                nc=nc,
            )
            return tuple(outs)

        devices = jax.devices()[:n_cores]
        assert len(devices) == n_cores
        n_params = len(in_names)
        n_outs = len(out_names)
        donate = tuple(range(n_params, n_params + n_outs))
        if n_cores == 1:
            self._fn = jax.jit(_body, donate_argnums=donate, keep_unused=True)
            self._concat = False
        else:
            mesh = Mesh(np.asarray(devices), ("core",))
            self._fn = jax.jit(shard_map(
                _body, mesh=mesh,
                in_specs=(PartitionSpec("core"),) * (n_params + n_outs),
                out_specs=(PartitionSpec("core"),) * n_outs,
                check_rep=False), donate_argnums=donate, keep_unused=True)
            self._concat = True
        self._args = None

    def set_inputs(self, in_maps):
        assert len(in_maps) == self.n_cores
        args = []
        for name in self.in_names:
            if self._concat:
                args.append(np.concatenate(
                    [in_maps[c][name] for c in range(self.n_cores)], axis=0))
            else:
                args.append(in_maps[0][name])
        self._args = [self.jax.device_put(a) for a in args]
        self.jax.block_until_ready(self._args)

    def _zero_outs(self):
        import jax.numpy as jnp
        outs = []
        for av in self.out_avals:
            shape = (self.n_cores * av.shape[0], *av.shape[1:]) if self._concat else av.shape
            outs.append(jnp.zeros(shape, av.dtype))
        return outs

    def run(self):
        outs = self._fn(*self._args, *self._zero_outs())
        self.jax.block_until_ready(outs)
        return outs

    def results(self, outs):
        res = []
        for c in range(self.n_cores):
            d = {}
            for i, name in enumerate(self.out_names):
                a = np.asarray(outs[i])
                if self._concat:
                    a = a.reshape(self.n_cores, *self.out_avals[i].shape)[c]
                d[name] = a
            res.append(d)
        return res

    def time(self, iters=30, warmup=3):
        ts = []
        for _ in range(warmup):
            self.run()
        for _ in range(iters):
            t0 = time.perf_counter()
            self.run()
            ts.append(time.perf_counter() - t0)
        return float(np.median(ts)), float(np.min(ts))


# ---------------------------------------------------------------- entry

_CACHE = {}


def _get_runner(cfg):
    key = (cfg["N_LOC"], cfg["W"], cfg["T"])
    if key not in _CACHE:
        nc = _build(cfg)
        _CACHE[key] = SpmdRunner(nc, N_CORES)
    return _CACHE[key]


def _make_in_maps(cfg, xT, cores, w):
    in_maps = []
    for k in range(N_CORES):
        c = cores[k]
        in_maps.append({
            "xT": xT, "xTloc": c["xT_loc"], "eaT": c["eaT"],
            "rowidx": c["rowidx"], "colidx": c["colidx"],
            "colshift": c["colshift"], "degT": c["degT"],
            "wm1ab": w["wm1ab"], "wm1c": w["wm1c"], "wpb": w["wpb"],
            "wu1a": w["wu1a"], "wu2": w["wu2"], "vecs": w["vecs"],
        })
    return in_maps


def _assemble(cfg, results):
    full = np.empty((N_NODES, OUT_DIM), np.float32)
    nb, n_k = cfg["nb"], cfg["n_k"]
    for k in range(N_CORES):
        if n_k[k] > 0:
            full[nb[k]:nb[k] + n_k[k]] = results[k]["out"][:n_k[k]]
    return full


def kernel(**inputs):
    x = np.asarray(inputs["x"], np.float32)
    cfg, xT, cores = _host_prep(x, inputs["edge_index"], inputs["edge_attr"])
    w = _host_weights(inputs["Wm1"], inputs["bm1"], inputs["Wm2"], inputs["bm2"],
                      inputs["Wu1"], inputs["bu1"], inputs["Wu2"], inputs["bu2"])
    runner = _get_runner(cfg)
    runner.set_inputs(_make_in_maps(cfg, xT, cores, w))
    outs = runner.run()
    return _assemble(cfg, runner.results(outs))
